# revision 24
# baseline (speedup 1.0000x reference)
"""Trainium2 Bass kernel for nn_BiAttnGRUEncoder.

Data-parallel over batch (B=64 -> 8 cores x b=8). Each core runs the full
model on its batch slice: embedding gather, 2-layer bidirectional GRU
(4 recurrent chains), and the attention head. No cross-core communication.

Layouts are feature-major ("transposed") throughout so the matmul
contraction dim always sits on SBUF partitions:
  x^T       (384+4, tokens)     tokens ordered t-major, b-minor
  gi        (12, 128, T, b)     gate-chunk-major input contributions (DRAM)
  h archive (128, 4, T, b)      recurrent states, H-major (SBUF resident)
  attn out  (1024, T, b)        host transposes back
"""

import sys
import os

sys.path.insert(0, "/opt/trn_rl_repo")

import numpy as np
import ml_dtypes

import concourse.bass as bass
from concourse import mybir
from concourse.bass_utils import run_bass_kernel_spmd
from concourse.tile import TileContext
from concourse.vector_clock import ScopedClock

BF16 = mybir.dt.bfloat16
F32 = mybir.dt.float32
I32 = mybir.dt.int32
AF = mybir.ActivationFunctionType
ALU = mybir.AluOpType

V, E, INP, H = 50000, 300, 303, 512
G3 = 3 * H  # 1536
NCORES = 8
LAST_RESULT = None


# ---------------------------------------------------------------------------
# Tile tail-drain workaround: walrus in this toolchain rejects >1 sync wait
# on the final Drain CTRL instruction. Split the waits into single-wait SP
# nops emitted just before the drain.
# ---------------------------------------------------------------------------
def _patched_drain_and_barrier(self, tick_clock, wait_clock):
    nc = self.nc
    probe = nc.sync.nop(nofuse=True)
    wait_clock.add_sem_waits(probe.ins, ScopedClock({None: tick_clock.global_clock}))
    si = probe.ins.sync_info
    waits = list(si.on_wait or []) if si is not None else []
    if len(waits) > 1:
        si.on_wait.clear()
        si.on_wait.append(waits[0])
        for w in waits[1:]:
            n = nc.sync.nop(nofuse=True)
            n.ins.sync_info = mybir.SyncInfo(on_wait=[w], on_update=[])
    drain_inst = nc.sync.drain()
    dsi = drain_inst.ins.sync_info
    if dsi is not None and dsi.on_wait:
        dsi.on_wait.clear()
    nc.all_engine_barrier()
    assert self.sems is not None
    popped = nc._tile_sem_poison_stack.pop()
    assert popped is self._sem_poison
    nc.clear_and_free_semaphores(list(self.sems.allocated().values()))
    nc.all_engine_barrier()


TileContext._drain_and_barrier = _patched_drain_and_barrier


def _split_sync_waits(nc):
    """walrus in this toolchain accepts at most one sync-wait per
    instruction. Hoist extra waits onto single-wait nops placed directly
    before the instruction, on the same engine queue."""
    for fn in nc.m.functions:
        for bb in fn.blocks:
            insts = list(bb.instructions)
            out = []
            changed = False
            for inst in insts:
                si = getattr(inst, "sync_info", None)
                waits = list(si.on_wait) if (si is not None and si.on_wait) else []
                if len(waits) > 1:
                    changed = True
                    eng = nc.engines[inst.engine]
                    for w in waits[:-1]:
                        nop = eng.nop(nofuse=True)
                        # nop() appended itself to some block's tail; pop it
                        # by identity (never use `in`/`remove`: those call
                        # expensive structural __eq__ on Rust instructions).
                        for bb2 in fn.blocks:
                            lst = bb2.instructions
                            if len(lst) and lst[-1] is nop.ins:
                                lst.pop()
                                break
                        else:
                            raise RuntimeError("split nop not found at a tail")
                        nop.ins.sync_info = mybir.SyncInfo(on_wait=[w], on_update=[])
                        out.append(nop.ins)
                    si.on_wait.clear()
                    si.on_wait.append(waits[-1])
                out.append(inst)
            if changed:
                bb.instructions.clear()
                for i_ in out:
                    bb.instructions.append(i_)


def _bf16(x):
    return np.asarray(x, dtype=np.float32).astype(ml_dtypes.bfloat16)


# ---------------------------------------------------------------------------
# Program builder (one SPMD program, same for all cores)
# ---------------------------------------------------------------------------
def build_program(T: int, b: int):
    NTOK = T * b
    NB = NTOK // 512          # number of 512-token blocks
    NG = NTOK // 128          # number of 128-token gather chunks
    SCH = 8                   # gi staging chunk (steps)
    QB = T // 64              # query blocks (512 tokens each)

    nc = bass.Bass("TRN2")
    build_body(nc, T, b)
    _split_sync_waits(nc)
    return nc


def build_body(nc, T: int, b: int):
    NTOK = T * b
    NB = NTOK // 512          # number of 512-token blocks
    NG = NTOK // 128          # number of 128-token gather chunks
    SCH = 8                   # gi staging chunk (steps)
    QB = T // 64              # query blocks (512 tokens each)

    dp = lambda n, shp, dt: nc.declare_dram_parameter(n, shp, dt, isOutput=False)
    glove_d = dp("glove_pad", [V, 384], BF16)
    idx_d = dp("idx", [128, NG], I32)
    onehot_d = dp("onehot", [4, NTOK], BF16)
    ident_d = dp("ident", [128, 128], BF16)
    wih1_d = {s: dp(f"wih1_{s}", [388, G3], BF16) for s in "fb"}
    whh1_d = {s: dp(f"whh1_{s}", [H, G3], BF16) for s in "fb"}
    wih2_d = {s: dp(f"wih2_{s}", [H, G3], BF16) for s in "fb"}
    whh2_d = {s: dp(f"whh2_{s}", [H, G3], BF16) for s in "fb"}
    wlin_d = dp("wlinT", [1024, 1024], BF16)
    wg_d = dp("wgT", [2048, 1024], BF16)
    wf_d = dp("wfT", [2048, 1024], BF16)

    conc_o = {s: nc.declare_dram_parameter(f"conc_{s}", [128, 4, b], BF16, isOutput=True)
              for s in "fb"}
    attn_o = nc.declare_dram_parameter("attn_T", [1024, T, b], F32, isOutput=True)

    with TileContext(nc) as tc:
        with (
            tc.tile_pool(name="const", bufs=1) as constp,
            tc.tile_pool(name="dram", bufs=1, space="DRAM") as dramp,
        ):
            ident = constp.tile([128, 128], BF16)
            nc.sync.dma_start(out=ident, in_=ident_d[:, :])
            idx_sb = constp.tile([128, NG], I32)
            nc.sync.dma_start(out=idx_sb, in_=idx_d[:, :])
            onehot_sb = constp.tile([4, NTOK], BF16)
            nc.sync.dma_start(out=onehot_sb, in_=onehot_d[:, :])
            zerostate = constp.tile([128, 4, b], BF16)
            nc.vector.memset(zerostate, 0.0)

            gi1 = {s: dramp.tile([12, 128, T, b], BF16, tag=f"gi1{s}", name=f"gi1{s}") for s in "fb"}
            gi2 = {s: dramp.tile([12, 128, T, b], BF16, tag=f"gi2{s}", name=f"gi2{s}") for s in "fb"}
            ctx_dram = dramp.tile([128, 8, T * b], BF16, tag="ctxT", name="ctxT")

            # h1 archives live until gi2 is done; allocated below everything
            # that gets freed earlier (LIFO pool allocator).
            with tc.tile_pool(name="arch1", bufs=1) as arch1p:
                h1 = {s: arch1p.tile([128, 4, T, b], BF16, tag=f"h1{s}", name=f"h1{s}") for s in "fb"}

                # ---------------- phase A: gi1 + layer-1 recurrence --------
                with tc.tile_pool(name="w1", bufs=1) as w1p:
                    wih1 = {}
                    whh1 = {}
                    for s in "fb":
                        t_ = w1p.tile([128, 4, G3], BF16, tag=f"wih1{s}", name=f"wih1{s}")
                        nc.sync.dma_start(
                            out=t_[:, 0:3, :],
                            in_=wih1_d[s][0:384, :].rearrange("(k p) n -> p k n", p=128),
                        )
                        nc.sync.dma_start(out=t_[0:4, 3, :], in_=wih1_d[s][384:388, :])
                        wih1[s] = t_
                        t2 = w1p.tile([128, 4, G3], BF16, tag=f"whh1{s}", name=f"whh1{s}")
                        nc.sync.dma_start(
                            out=t2,
                            in_=whh1_d[s][:, :].rearrange("(k p) n -> p k n", p=128),
                        )
                        whh1[s] = t2

                    # ---- gi1: gather + transpose + input matmuls ----
                    with (
                        nc.named_scope("gi1"),
                        tc.tile_pool(name="giwork", bufs=3) as gw,
                        tc.tile_pool(name="gixt", bufs=2) as gxt,
                        tc.tile_pool(name="gipsum", bufs=2, space="PSUM") as gps,
                        tc.tile_pool(name="gitps", bufs=2, space="PSUM") as tps,
                    ):
                        for nb in range(NB):
                            xT = gxt.tile([128, 3, 512], BF16, tag="xT")
                            for g in range(4):
                                c = nb * 4 + g
                                xg = gw.tile([128, 384], BF16, tag="xg")
                                nc.gpsimd.indirect_dma_start(
                                    out=xg[:, :],
                                    out_offset=None,
                                    in_=glove_d[:, :],
                                    in_offset=bass.IndirectOffsetOnAxis(
                                        ap=idx_sb[:, c:c + 1], axis=0
                                    ),
                                )
                                for k in range(3):
                                    pt = tps.tile([128, 128], BF16, tag="tp")
                                    nc.tensor.transpose(
                                        out=pt[:, :],
                                        in_=xg[:, k * 128:(k + 1) * 128],
                                        identity=ident[:, :],
                                    )
                                    nc.vector.tensor_copy(
                                        out=xT[:, k, g * 128:(g + 1) * 128], in_=pt
                                    )
                            for s in "fb":
                                for m in range(12):
                                    pg = gps.tile([128, 512], F32, tag="pg")
                                    for k in range(3):
                                        nc.tensor.matmul(
                                            out=pg[:, :],
                                            lhsT=wih1[s][:, k, m * 128:(m + 1) * 128],
                                            rhs=xT[:, k, :],
                                            start=(k == 0),
                                            stop=False,
                                        )
                                    nc.tensor.matmul(
                                        out=pg[:, :],
                                        lhsT=wih1[s][0:4, 3, m * 128:(m + 1) * 128],
                                        rhs=onehot_sb[:, nb * 512:(nb + 1) * 512],
                                        start=False,
                                        stop=True,
                                    )
                                    ev = gw.tile([128, 512], BF16, tag="ev")
                                    if m % 2 == 0:
                                        nc.scalar.copy(out=ev, in_=pg)
                                    else:
                                        nc.vector.tensor_copy(out=ev, in_=pg)
                                    nc.sync.dma_start(
                                        out=gi1[s][m, :, nb * 64:(nb + 1) * 64, :],
                                        in_=ev.rearrange("p (t c) -> p t c", c=b),
                                    )

                    # ---- layer-1 recurrence (f and b interleaved) ----
                    with nc.named_scope("recA"):
                        _recurrence(nc, tc, T, b, SCH, whh1, gi1, h1, zerostate,
                                    rev={"f": False, "b": True}, scale_h=None)

                # ---- gi2 from h1 archives ----
                with (
                    nc.named_scope("gi2"),
                    tc.tile_pool(name="w2a", bufs=1) as w2ap,
                    tc.tile_pool(name="gi2work", bufs=3) as g2w,
                    tc.tile_pool(name="gi2psum", bufs=2, space="PSUM") as g2ps,
                ):
                    wih2 = {}
                    for s in "fb":
                        t_ = w2ap.tile([128, 4, G3], BF16, tag=f"wih2{s}", name=f"wih2{s}")
                        nc.sync.dma_start(
                            out=t_,
                            in_=wih2_d[s][:, :].rearrange("(k p) n -> p k n", p=128),
                        )
                        wih2[s] = t_
                    for s in "fb":
                        for nb in range(NB):
                            for m in range(12):
                                pg = g2ps.tile([128, 512], F32, tag="pg2")
                                for k in range(4):
                                    nc.tensor.matmul(
                                        out=pg[:, :],
                                        lhsT=wih2[s][:, k, m * 128:(m + 1) * 128],
                                        rhs=h1[s][:, k, nb * 64:(nb + 1) * 64, :],
                                        start=(k == 0),
                                        stop=(k == 3),
                                    )
                                ev = g2w.tile([128, 512], BF16, tag="ev2")
                                if m % 2 == 0:
                                    nc.scalar.copy(out=ev, in_=pg)
                                else:
                                    nc.vector.tensor_copy(out=ev, in_=pg)
                                nc.sync.dma_start(
                                    out=gi2[s][m, :, nb * 64:(nb + 1) * 64, :],
                                    in_=ev.rearrange("p (t c) -> p t c", c=b),
                                )

                # layer-1 final states -> output
                for s in "fb":
                    nc.sync.dma_start(out=conc_o[s][:, :, :], in_=h1[s][:, :, T - 1, :])

            # ---------------- phase B: layer-2 recurrence ------------------
            with tc.tile_pool(name="arch2", bufs=1) as arch2p:
                h2 = {s: arch2p.tile([128, 4, T, b], BF16, tag=f"h2{s}", name=f"h2{s}") for s in "fb"}
                with tc.tile_pool(name="w2b", bufs=1) as w2bp:
                    whh2 = {}
                    for s in "fb":
                        t2 = w2bp.tile([128, 4, G3], BF16, tag=f"whh2{s}", name=f"whh2{s}")
                        nc.sync.dma_start(
                            out=t2,
                            in_=whh2_d[s][:, :].rearrange("(k p) n -> p k n", p=128),
                        )
                        whh2[s] = t2
                    with nc.named_scope("recB"):
                        _recurrence(nc, tc, T, b, SCH, whh2, gi2, h2, zerostate,
                                    rev={"f": False, "b": False}, scale_h=None)

                # ---------------- phase C pass 1: q, scores, softmax, ctx --
                with (
                    nc.named_scope("attn1"),
                    tc.tile_pool(name="kmajp", bufs=1) as kmajp,
                    tc.tile_pool(name="wlinp", bufs=1) as wlinp,
                    tc.tile_pool(name="a1work", bufs=3) as a1w,
                    tc.tile_pool(name="a1small", bufs=4) as a1s,
                    tc.tile_pool(name="qtpool", bufs=2) as qtp,
                    tc.tile_pool(name="ps_qt", bufs=2, space="PSUM") as ps_qt,
                    tc.tile_pool(name="ps_sc", bufs=2, space="PSUM") as ps_sc,
                    tc.tile_pool(name="ps_tr", bufs=2, space="PSUM") as ps_tr,
                    tc.tile_pool(name="ps_cx", bufs=2, space="PSUM") as ps_cx,
                ):
                    KT = T // 128
                    wlin = wlinp.tile([128, 8, 1024], BF16)
                    nc.sync.dma_start(
                        out=wlin, in_=wlin_d[:, :].rearrange("(k p) n -> p k n", p=128)
                    )
                    # kmaj: token-major all_h per batch item (for ctx matmuls)
                    kmaj = kmajp.tile([128, KT, 1024, b], BF16)
                    for si, s in enumerate("fb"):
                        for bi in range(b):
                            for kc in range(4):
                                for tb in range(KT):
                                    pt = ps_tr.tile([128, 128], BF16, tag="trp")
                                    nc.tensor.transpose(
                                        out=pt[:, :],
                                        in_=h2[s][:, kc, tb * 128:(tb + 1) * 128, bi],
                                        identity=ident[:, :],
                                    )
                                    dcol = si * 512 + kc * 128
                                    nc.vector.tensor_copy(
                                        out=kmaj[:, tb, dcol:dcol + 128, bi], in_=pt
                                    )

                    def arch_slice(kc, t0, t1):
                        s = "f" if kc < 4 else "b"
                        return h2[s][:, kc % 4, t0:t1, :]

                    for qb in range(QB):
                        t0, t1 = qb * 64, (qb + 1) * 64
                        ctxT_sb = qtp.tile([128, 8, 512], BF16, tag="cxT")
                        ctxT_v = ctxT_sb.rearrange("p d (t c) -> p d t c", c=b)
                        # q^T for this query block
                        qT = qtp.tile([128, 8, 512], BF16, tag="qT")
                        for dc in range(8):
                            pq = ps_qt.tile([128, 512], F32, tag="pq")
                            for kc in range(8):
                                nc.tensor.matmul(
                                    out=pq[:, :],
                                    lhsT=wlin[:, kc, dc * 128:(dc + 1) * 128],
                                    rhs=arch_slice(kc, t0, t1),
                                    start=(kc == 0),
                                    stop=(kc == 7),
                                )
                            if dc % 2 == 0:
                                nc.scalar.copy(out=qT[:, dc, :], in_=pq)
                            else:
                                nc.vector.tensor_copy(out=qT[:, dc, :], in_=pq)
                        qTr = qT.rearrange("p d (t c) -> p d t c", c=b)
                        for bg in range(b // 2):
                            sc = ps_sc.tile([128, T], F32, tag="sc")
                            for bi2 in range(2):
                                bi = bg * 2 + bi2
                                for kc in range(8):
                                    s_ = "f" if kc < 4 else "b"
                                    nc.tensor.matmul(
                                        out=sc[bi2 * 64:(bi2 + 1) * 64, :],
                                        lhsT=qTr[:, kc, :, bi],
                                        rhs=h2[s_][:, kc % 4, :, bi],
                                        start=(kc == 0),
                                        stop=(kc == 7),
                                    )
                            nmax = a1s.tile([128, 1], F32, tag="nmax")
                            nc.vector.tensor_reduce(
                                out=nmax, in_=sc, axis=mybir.AxisListType.X,
                                op=ALU.max, negate=True,
                            )
                            ex = a1w.tile([128, T], BF16, tag="ex")
                            nc.scalar.activation(out=ex, in_=sc, func=AF.Exp,
                                                 bias=nmax, scale=1.0)
                            ssum = a1s.tile([128, 1], F32, tag="ssum")
                            nc.vector.tensor_reduce(
                                out=ssum, in_=ex, axis=mybir.AxisListType.X, op=ALU.add
                            )
                            rsum = a1s.tile([128, 1], F32, tag="rsum")
                            nc.vector.reciprocal(out=rsum, in_=ssum)
                            p_ = a1w.tile([128, T], BF16, tag="p_")
                            nc.scalar.mul(out=p_, in_=ex, mul=rsum)
                            pT = a1w.tile([128, KT, 128], BF16, tag="pT")
                            for kt in range(KT):
                                pt = ps_tr.tile([128, 128], BF16, tag="trp")
                                nc.tensor.transpose(
                                    out=pt[:, :],
                                    in_=p_[:, kt * 128:(kt + 1) * 128],
                                    identity=ident[:, :],
                                )
                                nc.vector.tensor_copy(out=pT[:, kt, :], in_=pt)
                            for bi2 in range(2):
                                bi = bg * 2 + bi2
                                pcx = ps_cx.tile([128, 8, 64], F32, tag="pcx")
                                for dc in range(8):
                                    for kt in range(KT):
                                        nc.tensor.matmul(
                                            out=pcx[:, dc, :],
                                            lhsT=kmaj[:, kt, dc * 128:(dc + 1) * 128, bi],
                                            rhs=pT[:, kt, bi2 * 64:(bi2 + 1) * 64],
                                            start=(kt == 0),
                                            stop=(kt == KT - 1),
                                        )
                                if bi % 2 == 0:
                                    nc.scalar.copy(out=ctxT_v[:, :, :, bi], in_=pcx)
                                else:
                                    nc.vector.tensor_copy(out=ctxT_v[:, :, :, bi],
                                                          in_=pcx)
                        nc.sync.dma_start(
                            out=ctx_dram[:, :, qb * 512:(qb + 1) * 512], in_=ctxT_sb
                        )

                # -------- phase C pass 2: gates g/f and combine ------------
                with (
                    nc.named_scope("attn2"),
                    tc.tile_pool(name="wgf", bufs=1) as wgfp,
                    tc.tile_pool(name="a2work", bufs=3) as a2w,
                    tc.tile_pool(name="ps_gf", bufs=4, space="PSUM") as ps_gf,
                ):
                    wg_sb = wgfp.tile([128, 16, 1024], BF16, tag="wg")
                    nc.sync.dma_start(
                        out=wg_sb, in_=wg_d[:, :].rearrange("(k p) n -> p k n", p=128)
                    )
                    wf_sb = wgfp.tile([128, 16, 1024], BF16, tag="wf")
                    nc.sync.dma_start(
                        out=wf_sb, in_=wf_d[:, :].rearrange("(k p) n -> p k n", p=128)
                    )
                    for qb in range(QB):
                        t0, t1 = qb * 64, (qb + 1) * 64
                        cxt = a2w.tile([128, 8, 512], BF16, tag="cxt")
                        nc.sync.dma_start(
                            out=cxt, in_=ctx_dram[:, :, qb * 512:(qb + 1) * 512]
                        )
                        for dc in range(8):
                            pg_ = ps_gf.tile([128, 512], F32, tag="pgg")
                            pf_ = ps_gf.tile([128, 512], F32, tag="pgf")
                            for w_sb, po in ((wg_sb, pg_), (wf_sb, pf_)):
                                for k4 in range(16):
                                    rhs = (cxt[:, k4, :] if k4 < 8
                                           else arch_slice(k4 - 8, t0, t1))
                                    nc.tensor.matmul(
                                        out=po[:, :],
                                        lhsT=w_sb[:, k4, dc * 128:(dc + 1) * 128],
                                        rhs=rhs,
                                        start=(k4 == 0),
                                        stop=(k4 == 15),
                                    )
                            # g = sigmoid(x) = 0.5*(1+tanh(x/2)) -> tg = tanh(x/2)
                            tg = a2w.tile([128, 512], BF16, tag="tg")
                            nc.scalar.activation(out=tg, in_=pg_, func=AF.Tanh,
                                                 bias=0.0, scale=0.5)
                            tf = a2w.tile([128, 512], BF16, tag="tf")
                            nc.scalar.activation(out=tf, in_=pf_, func=AF.Tanh,
                                                 bias=0.0, scale=1.0)
                            # attn = g*f + (1-g)*a = 0.5*(f + a + tg*(f-a))
                            # (the 0.5 is applied host-side)
                            a_ = arch_slice(dc, t0, t1)
                            d_ = a2w.tile([128, 512], BF16, tag="d_")
                            nc.vector.tensor_tensor(out=d_, in0=tf, in1=a_,
                                                    op=ALU.subtract)
                            e_ = a2w.tile([128, 512], BF16, tag="e_")
                            nc.vector.tensor_mul(out=e_, in0=tg, in1=d_)
                            s_ = a2w.tile([128, 512], BF16, tag="s_")
                            nc.vector.tensor_add(out=s_, in0=tf, in1=a_)
                            o_ = a2w.tile([128, 512], F32, tag="o_")
                            nc.vector.tensor_add(out=o_, in0=e_, in1=s_)
                            nc.sync.dma_start(
                                out=attn_o[dc * 128:(dc + 1) * 128, t0:t1, :],
                                in_=o_.rearrange("p (t c) -> p t c", c=b),
                            )


def _recurrence(nc, tc, T, b, SCH, whh, gi, arch, zerostate, rev, scale_h):
    """Two interleaved GRU chains (dirs 'f' and 'b'). gi/arch keyed by dir.
    rev[s]: read gi in reversed time order (layer-1 backward chain)."""
    with (
        tc.tile_pool(name="recstage", bufs=2) as stp,
        tc.tile_pool(name="rectmp", bufs=3) as tmp,
        tc.tile_pool(name="recpsum", bufs=2, space="PSUM") as rps,
    ):
        def load_chunk(s, t_first):
            st = stp.tile([128, 12, SCH, b], BF16, tag=f"st{s}", name=f"st{s}")
            tin0 = (T - SCH - t_first) if rev[s] else t_first
            nc.sync.dma_start(
                out=st,
                in_=gi[s][:, :, tin0:tin0 + SCH, :].rearrange("m p t c -> p m t c"),
            )
            return st

        cur = {s: load_chunk(s, 0) for s in "fb"}
        nxt = {s: load_chunk(s, SCH) for s in "fb"} if T > SCH else {}
        for t in range(T):
            for s in "fb":
                if t % SCH == 0 and t > 0:
                    cur[s] = nxt[s]
                    if t + SCH < T:
                        nxt[s] = load_chunk(s, t + SCH)
                st = cur[s]
                col = (SCH - 1 - t % SCH) if rev[s] else (t % SCH)

                hprev = zerostate[:, :, :] if t == 0 else arch[s][:, :, t - 1, :]
                ps = rps.tile([128, 12, b], F32, tag=f"ps{s}")
                for m in range(12):
                    for k in range(4):
                        nc.tensor.matmul(
                            out=ps[:, m, :],
                            lhsT=whh[s][:, k, m * 128:(m + 1) * 128],
                            rhs=hprev[:, k, :],
                            start=(k == 0),
                            stop=(k == 3),
                        )
                gr, gz, gn = ps[:, 0:4, :], ps[:, 4:8, :], ps[:, 8:12, :]
                tr = tmp.tile([128, 4, b], BF16, tag=f"tr{s}")
                nc.vector.tensor_add(out=tr, in0=gr, in1=st[:, 0:4, col, :])
                r = tmp.tile([128, 4, b], BF16, tag=f"r{s}")
                nc.scalar.activation(out=r, in_=tr, func=AF.Sigmoid)
                tz = tmp.tile([128, 4, b], BF16, tag=f"tz{s}")
                nc.vector.tensor_add(out=tz, in0=gz, in1=st[:, 4:8, col, :])
                z = tmp.tile([128, 4, b], BF16, tag=f"z{s}")
                nc.scalar.activation(out=z, in_=tz, func=AF.Sigmoid)
                u = tmp.tile([128, 4, b], BF16, tag=f"u{s}")
                nc.vector.tensor_mul(out=u, in0=r, in1=gn)
                w_ = tmp.tile([128, 4, b], BF16, tag=f"w{s}")
                nc.vector.tensor_add(out=w_, in0=u, in1=st[:, 8:12, col, :])
                n_ = tmp.tile([128, 4, b], BF16, tag=f"n{s}")
                nc.scalar.activation(out=n_, in_=w_, func=AF.Tanh)
                d_ = tmp.tile([128, 4, b], BF16, tag=f"d{s}")
                nc.vector.tensor_tensor(out=d_, in0=hprev, in1=n_, op=ALU.subtract)
                e_ = tmp.tile([128, 4, b], BF16, tag=f"e{s}")
                nc.vector.tensor_mul(out=e_, in0=z, in1=d_)
                nc.vector.tensor_add(out=arch[s][:, :, t, :], in0=n_, in1=e_)


# ---------------------------------------------------------------------------
# Host side
# ---------------------------------------------------------------------------
def _prep_weights(inputs, T):
    """Build the per-core shared (weight) input map, all bf16."""
    f32 = lambda x: np.asarray(x, dtype=np.float32)
    glove = f32(inputs["glove"])
    bio = f32(inputs["bio"])
    gp = np.zeros((V, 384), dtype=np.float32)
    gp[:, :E] = glove
    shared = {"glove_pad": _bf16(gp), "ident": _bf16(np.eye(128))}

    for s, n1, n2 in (("f", "f1", "f2"), ("b", "b1", "b2")):
        wih = f32(inputs[f"Wih_{n1}"])          # (3H, INP)
        aug = np.zeros((388, G3), dtype=np.float32)
        aug[:E, :] = wih[:, :E].T
        aug[384:388, :] = bio @ wih[:, E:INP].T  # onehot rows (tags 0..3)
        # fold biases (zero in this problem, but keep exact for r/z/n input side)
        aug[384:388, :] += (f32(inputs[f"bih_{n1}"]) + f32(inputs[f"bhh_{n1}"]))[None, :]
        shared[f"wih1_{s}"] = _bf16(aug)
        shared[f"whh1_{s}"] = _bf16(f32(inputs[f"Whh_{n1}"]).T)
        w2 = f32(inputs[f"Wih_{n2}"]).T          # (H, 3H)
        b2 = (f32(inputs[f"bih_{n2}"]) + f32(inputs[f"bhh_{n2}"]))
        assert abs(b2).max() == 0.0, "nonzero GRU biases not folded"
        shared[f"wih2_{s}"] = _bf16(w2)
        shared[f"whh2_{s}"] = _bf16(f32(inputs[f"Whh_{n2}"]).T)

    shared["wlinT"] = _bf16(f32(inputs["W_lin"]).T)
    shared["wgT"] = _bf16(f32(inputs["W_g"]).T)
    shared["wfT"] = _bf16(f32(inputs["W_f"]).T)
    assert abs(f32(inputs["b_lin"])).max() == 0.0
    assert abs(f32(inputs["b_g"])).max() == 0.0
    assert abs(f32(inputs["b_f"])).max() == 0.0
    return shared


def _bench_run(nc, in_maps, n_cores, iters=3):
    """Mirror bass2jax.run_bass_via_pjrt's multi-core path, but keep inputs
    device-resident and time repeated executions (reports min wall per run)."""
    import time
    import jax
    from jax.sharding import Mesh, PartitionSpec
    from jax.experimental.shard_map import shard_map
    from concourse import bass2jax, mybir as _mb
    from concourse.bass2jax import _bass_exec_p, partition_id_tensor

    bass2jax.install_neuronx_cc_hook()
    partition_name = nc.partition_id_tensor.name if nc.partition_id_tensor else None
    in_names, out_names, out_avals, zero_outs = [], [], [], []
    for alloc in nc.m.functions[0].allocations:
        if not isinstance(alloc, _mb.MemoryLocationSet):
            continue
        name = alloc.memorylocations[0].name
        if alloc.kind == "ExternalInput":
            if name != partition_name:
                in_names.append(name)
        elif alloc.kind == "ExternalOutput":
            out_names.append(name)
            shape = tuple(alloc.tensor_shape)
            dtype = _mb.dt.np(alloc.dtype)
            out_avals.append(jax.core.ShapedArray(shape, dtype))
            zero_outs.append(np.zeros(shape, dtype))
    n_params = len(in_names)
    n_outs = len(out_avals)
    in_names_all = in_names + out_names + ([partition_name] if partition_name else [])
    donate = tuple(range(n_params, n_params + n_outs))

    def _body(*args):
        operands = list(args)
        if partition_name is not None:
            operands.append(partition_id_tensor())
        outs = _bass_exec_p.bind(
            *operands,
            out_avals=tuple(out_avals),
            in_names=tuple(in_names_all),
            out_names=tuple(out_names),
            lowering_input_output_aliases=(),
            sim_require_finite=True,
            sim_require_nnan=True,
            nc=nc,
        )
        return tuple(outs)

    devices = jax.devices()[:n_cores]
    mesh = Mesh(np.asarray(devices), ("core",))
    in_specs = (PartitionSpec("core"),) * (n_params + n_outs)
    out_specs = (PartitionSpec("core"),) * len(out_names)
    sharded = jax.jit(
        shard_map(_body, mesh=mesh, in_specs=in_specs, out_specs=out_specs,
                  check_rep=False),
        donate_argnums=donate, keep_unused=True,
    )
    concat_in = [
        np.concatenate([np.asarray(in_maps[c][nm]) for c in range(n_cores)], axis=0)
        for nm in in_names
    ]
    concat_zeros = [
        np.zeros((n_cores * z.shape[0], *z.shape[1:]), z.dtype) for z in zero_outs
    ]
    from jax.sharding import NamedSharding
    shard = NamedSharding(mesh, PartitionSpec("core"))
    dev_in = [jax.device_put(x, shard) for x in concat_in]
    best = None
    out_arrs = None
    for it in range(iters):
        dev_zero = [jax.device_put(z, shard) for z in concat_zeros]
        jax.block_until_ready(dev_zero)
        jax.block_until_ready(dev_in)
        t0 = time.perf_counter()
        out_arrs = sharded(*dev_in, *dev_zero)
        jax.block_until_ready(out_arrs)
        dt = time.perf_counter() - t0
        print(f"  bench iter {it}: {dt*1e3:.2f} ms")
        best = dt if best is None else min(best, dt)
    global LAST_EXEC_NS
    LAST_EXEC_NS = int(best * 1e9)
    results = [
        {name: np.asarray(out_arrs[i]).reshape(n_cores, *out_avals[i].shape)[c]
         for i, name in enumerate(out_names)}
        for c in range(n_cores)
    ]
    return results


LAST_EXEC_NS = None


def kernel(**inputs):
    context = np.asarray(inputs["context"])
    tags = np.asarray(inputs["answer_tags"])
    B, T = context.shape[0], context.shape[1]
    b = B // NCORES
    NTOK = T * b

    shared = _prep_weights(inputs, T)
    nc = build_program(T, b)

    in_maps = []
    for c in range(NCORES):
        ctx_c = context[c * b:(c + 1) * b, :, 0].astype(np.int64)   # (b, T)
        tag_c = tags[c * b:(c + 1) * b, :].astype(np.int64)
        # token order: t-major, b-minor
        idx_flat = ctx_c.T.reshape(-1).astype(np.int32)             # (T*b,)
        idx_tile = idx_flat.reshape(NTOK // 128, 128).T.copy()      # (128, NG)
        tag_flat = tag_c.T.reshape(-1)
        onehot = np.zeros((4, NTOK), dtype=np.float32)
        onehot[tag_flat, np.arange(NTOK)] = 1.0
        m = dict(shared)
        m["idx"] = idx_tile
        m["onehot"] = _bf16(onehot)
        in_maps.append(m)

    bench = bool(int(os.environ.get("KERNEL_BENCH", "0")))
    if bench:
        results = _bench_run(nc, in_maps, NCORES)
    else:
        res = run_bass_kernel_spmd(nc, in_maps, list(range(NCORES)))
        global LAST_RESULT
        LAST_RESULT = res
        results = res.results

    conc = np.zeros((B, 2 * H), dtype=np.float32)
    attn = np.zeros((B, T, 2 * H), dtype=np.float32)
    for c in range(NCORES):
        r = results[c]
        cf = np.asarray(r["conc_f"], dtype=np.float32)   # (128, 4, b)
        cb = np.asarray(r["conc_b"], dtype=np.float32)
        conc[c * b:(c + 1) * b, :H] = cf.transpose(2, 1, 0).reshape(b, H)
        conc[c * b:(c + 1) * b, H:] = cb.transpose(2, 1, 0).reshape(b, H)
        at = np.asarray(r["attn_T"], dtype=np.float32)   # (1024, T, b)
        attn[c * b:(c + 1) * b] = 0.5 * at.transpose(2, 1, 0)
    return conc, attn
